# revision 6
# baseline (speedup 1.0000x reference)
"""Performer/FAVOR+ linear attention on 8 Trainium2 NeuronCores.

out = phiQ.T @ (phiK @ V),  phi(X) = D_norm * exp(A + B*(W@X) + c*|x|^2)

Sharding: sequence axis L=65536 split across 8 cores (8192 each). Each core:
  K-side: per 128-col chunk, compute (W@K_chunk)^T in PSUM via
    matmul(lhsT=K_chunk, rhs=W^T), exp with per-partition bias
    (A + ln D_norm - |k|^2) -> phiK^T [128, 256]; accumulate
    kv^T [64, 256] += V_chunk^T @ phiK^T in PSUM over all 64 chunks.
  AllReduce kv^T (64KB) across the 8 cores (overlapped with Q-side features).
  Q-side: rawQ = exp(A + ln D_norm + B*(W@Q)) [256, L] with W^T stationary
    (bf16 output), |q|^2 via ones-matmul; out_chunk [128, 64] =
    rawQ_chunk^T @ kv, scaled per-partition by exp(-|q|^2).
"""
import numpy as np

N_CORES = 8
D = 64
L = 65536
R = 256
DV = 64
LS = L // N_CORES          # 8192 per core
TILE = 2048                # streaming tile of L
N_TILES = LS // TILE       # 4
CPT = TILE // 128          # chunks per tile = 16
N_CHUNKS = LS // 128       # 64
QB = 512                   # q feature block
N_QB = LS // QB            # 16

A_CONST = 0.1
S_CONST = 1.0
B_CONST = (S_CONST * (1.0 - 4.0 * A_CONST)) ** 0.5
LN_DNORM = (D / 4.0) * float(np.log(1.0 - 4.0 * A_CONST))
BIAS_CONST = A_CONST + LN_DNORM          # exp bias for both sides
# both phi calls use c = -1.0 (C_CONST*S_CONST and C_CONST)

_CACHE = {}


def _build():
    import concourse.bass as bass
    import concourse.bacc as bacc
    import concourse.tile as tile
    import concourse.mybir as mybir

    f32 = mybir.dt.float32
    f32r = mybir.dt.float32r
    bf16 = mybir.dt.bfloat16

    nc = bacc.Bacc("TRN2", target_bir_lowering=False, debug=False,
                   num_devices=N_CORES)
    k_ext = nc.dram_tensor("K", [D, LS], f32r, kind="ExternalInput").ap()
    q_ext = nc.dram_tensor("Q", [D, LS], f32r, kind="ExternalInput").ap()
    v_ext = nc.dram_tensor("Vr", [128, N_CHUNKS * DV], f32r,
                           kind="ExternalInput").ap()
    wt_ext = nc.dram_tensor("Wt", [D, R + 2], f32r, kind="ExternalInput").ap()
    i64_ext = nc.dram_tensor("I64", [64, 64], f32, kind="ExternalInput").ap()
    out_ext = nc.dram_tensor("out", [LS, DV], f32, kind="ExternalOutput").ap()

    with tile.TileContext(nc) as tc:
        with (
            tc.tile_pool(name="const", bufs=1) as cpool,
            tc.tile_pool(name="kq", bufs=3) as kq_pool,
            tc.tile_pool(name="x2", bufs=2) as x2_pool,
            tc.tile_pool(name="phikt", bufs=4) as phikt_pool,
            tc.tile_pool(name="rawq", bufs=2 * N_QB) as rawq_pool,
            tc.tile_pool(name="small", bufs=10) as small_pool,
            tc.tile_pool(name="outsb", bufs=4) as out_pool,
            tc.tile_pool(name="ps_feat", bufs=2, space="PSUM") as ps_feat,
            tc.tile_pool(name="ps_q", bufs=2, space="PSUM") as ps_q,
            tc.tile_pool(name="ps_kvt", bufs=1, space="PSUM") as ps_kvt,
            tc.tile_pool(name="ps_sq", bufs=1, space="PSUM") as ps_sq,
            tc.tile_pool(name="ps_out", bufs=2, space="PSUM") as ps_out,
            tc.tile_pool(name="dram", bufs=1, space="DRAM") as dram,
        ):
            # constants
            wt_sb = cpool.tile([D, R + 2], f32r, name="wt_sb")
            nc.sync.dma_start(wt_sb[:], wt_ext[:])
            i64_sb = cpool.tile([64, 64], f32, name="i64_sb")
            nc.sync.dma_start(i64_sb[:], i64_ext[:])
            biasq_sb = cpool.tile([128, 1], f32, name="biasq_sb")
            nc.vector.memset(biasq_sb[:], BIAS_CONST)
            zero_sb = cpool.tile([128, 1], f32, name="zero_sb")
            nc.vector.memset(zero_sb[:], 0.0)
            v_sb = cpool.tile([128, N_CHUNKS * DV], f32r, name="v_sb")
            nc.sync.dma_start(v_sb[:], v_ext[:])

            wt_r = wt_sb[:, 0:R]
            ones_r = wt_sb[:, R:R + 2]

            kvt_ps = ps_kvt.tile([64, R], f32)   # kv^T accumulator

            # ---------------- K side ----------------
            for t in range(N_TILES):
                k_t = kq_pool.tile([D, TILE], f32r, tag="kq")
                nc.sync.dma_start(k_t[:], k_ext[:, t * TILE:(t + 1) * TILE])
                k2_t = x2_pool.tile([D, TILE], f32r, tag="x2")
                nc.vector.tensor_mul(k2_t[:], k_t[:], k_t[:])
                sq_ps = ps_sq.tile([128, 2 * CPT], f32, tag="sq")
                for j in range(CPT):
                    nc.tensor.matmul(
                        sq_ps[:, 2 * j:2 * j + 2],
                        k2_t[:, j * 128:(j + 1) * 128],
                        ones_r,
                        start=True, stop=True,
                    )
                sq_view = sq_ps[:].rearrange("p (c two) -> p c two", two=2)[:, :, 0]
                bias_t = small_pool.tile([128, CPT], f32, tag="biask")
                nc.vector.tensor_scalar(
                    bias_t[:], sq_view, -1.0, BIAS_CONST,
                    mybir.AluOpType.mult, mybir.AluOpType.add,
                )
                for j in range(CPT):
                    i = t * CPT + j
                    feat_ps = ps_feat.tile([128, R], f32, tag="feat")
                    nc.tensor.matmul(
                        feat_ps[:],
                        k_t[:, j * 128:(j + 1) * 128],
                        wt_r,
                        start=True, stop=True,
                    )
                    phikt = phikt_pool.tile([128, R], f32r, tag="phikt")
                    nc.scalar.activation(
                        phikt[:], feat_ps[:],
                        mybir.ActivationFunctionType.Exp,
                        bias=bias_t[:, j:j + 1], scale=B_CONST,
                    )
                    nc.tensor.matmul(
                        kvt_ps[:],
                        v_sb[:, i * DV:(i + 1) * DV],
                        phikt[:],
                        start=(i == 0), stop=(i == N_CHUNKS - 1),
                    )

            # ---------------- kv^T AllReduce ----------------
            kvt_sb = cpool.tile([64, R], f32, name="kvt_sb")
            nc.vector.tensor_copy(kvt_sb[:], kvt_ps[:])
            cc_in = dram.tile([64, R], f32)
            cc_out = dram.tile([64, R], f32)
            nc.sync.dma_start(cc_in[:], kvt_sb[:])
            nc.gpsimd.collective_compute(
                "AllReduce",
                mybir.AluOpType.add,
                ins=[cc_in.opt()],
                outs=[cc_out.opt()],
                replica_groups=[list(range(N_CORES))],
            )

            # ---------------- Q side features (overlap collective) --------
            rawq = []   # [h][b] -> [128, 512] bf16
            sq_q_tiles = []
            for t in range(N_TILES):
                q_t = kq_pool.tile([D, TILE], f32r, tag="kq")
                nc.sync.dma_start(q_t[:], q_ext[:, t * TILE:(t + 1) * TILE])
                q2_t = x2_pool.tile([D, TILE], f32r, tag="x2")
                nc.vector.tensor_mul(q2_t[:], q_t[:], q_t[:])
                sq_ps = ps_sq.tile([128, 2 * CPT], f32, tag="sq")
                for j in range(CPT):
                    nc.tensor.matmul(
                        sq_ps[:, 2 * j:2 * j + 2],
                        q2_t[:, j * 128:(j + 1) * 128],
                        ones_r,
                        start=True, stop=True,
                    )
                sq_view = sq_ps[:].rearrange("p (c two) -> p c two", two=2)[:, :, 0]
                sq_t = small_pool.tile([128, CPT], f32, tag="sqq")
                nc.scalar.activation(
                    sq_t[:], sq_view,
                    mybir.ActivationFunctionType.Exp,
                    bias=zero_sb[:], scale=-1.0,
                )
                sq_q_tiles.append(sq_t)
                for b in range(TILE // QB):
                    for h in range(2):
                        q_ps = ps_q.tile([128, QB], f32, tag="q")
                        nc.tensor.matmul(
                            q_ps[:],
                            wt_sb[:, h * 128:(h + 1) * 128],
                            q_t[:, b * QB:(b + 1) * QB],
                            start=True, stop=True,
                        )
                        rq = rawq_pool.tile([128, QB], bf16, tag="rawq")
                        nc.scalar.activation(
                            rq[:], q_ps[:],
                            mybir.ActivationFunctionType.Exp,
                            bias=biasq_sb[:], scale=B_CONST,
                        )
                        rawq.append(rq)

            # ---------------- kv back + transpose ----------------
            kvt_red = cpool.tile([64, R], f32, name="kvt_red")
            nc.sync.dma_start(kvt_red[:], cc_out[:])
            kv_bf = []
            for h in range(2):
                kv_ps = ps_out.tile([128, DV], f32, tag="out")
                nc.tensor.transpose(
                    kv_ps[:], kvt_red[:, h * 128:(h + 1) * 128], i64_sb[:])
                kvh = cpool.tile([128, DV], bf16, name=f"kv_bf{h}")
                nc.vector.tensor_copy(kvh[:], kv_ps[:])
                kv_bf.append(kvh)

            # ---------------- out = rawQ^T @ kv, scaled ----------------
            for i in range(N_CHUNKS):
                gb = i // 4              # global 512-block
                s = (i % 4) * 128
                o_ps = ps_out.tile([128, DV], f32, tag="out")
                nc.tensor.matmul(
                    o_ps[:], rawq[2 * gb][:, s:s + 128], kv_bf[0][:],
                    start=True, stop=False,
                )
                nc.tensor.matmul(
                    o_ps[:], rawq[2 * gb + 1][:, s:s + 128], kv_bf[1][:],
                    start=False, stop=True,
                )
                o_sb = out_pool.tile([128, DV], f32, tag="osb")
                sq_t = sq_q_tiles[i // CPT]
                nc.vector.tensor_scalar(
                    o_sb[:], o_ps[:], sq_t[:, (i % CPT):(i % CPT) + 1], None,
                    mybir.AluOpType.mult,
                )
                nc.sync.dma_start(
                    out_ext[i * 128:(i + 1) * 128, :], o_sb[:])

    nc.compile()
    return nc


def _get_nc():
    if "nc" not in _CACHE:
        _CACHE["nc"] = _build()
    return _CACHE["nc"]


def _make_in_maps(Q, K, V, W):
    wt = np.ascontiguousarray(
        np.concatenate([W.T, np.ones((D, 2))], axis=1).astype(np.float32))
    i64 = np.eye(64, dtype=np.float32)
    maps = []
    for c in range(N_CORES):
        sl = slice(c * LS, (c + 1) * LS)
        vr = np.ascontiguousarray(
            V[sl].reshape(N_CHUNKS, 128, DV).transpose(1, 0, 2)
            .reshape(128, N_CHUNKS * DV).astype(np.float32))
        maps.append({
            "Q": np.ascontiguousarray(Q[:, sl].astype(np.float32)),
            "K": np.ascontiguousarray(K[:, sl].astype(np.float32)),
            "Vr": vr,
            "Wt": wt,
            "I64": i64,
        })
    return maps


def _run(in_maps):
    from concourse.bass_utils import run_bass_kernel_spmd
    nc = _get_nc()
    return run_bass_kernel_spmd(nc, in_maps, list(range(N_CORES)))


def kernel(Q, K, V, W):
    res = _run(_make_in_maps(Q, K, V, W))
    out = np.concatenate([res.results[c]["out"] for c in range(N_CORES)],
                         axis=0)
    return out.astype(np.float32)


# revision 8
# speedup vs baseline: 470.3400x; 470.3400x over previous
"""Performer/FAVOR+ linear attention on 8 Trainium2 NeuronCores.

out = phiQ.T @ (phiK @ V),  phi(X) = D_norm * exp(A + B*(W@X) + c*|x|^2)

Sharding: sequence axis L=65536 split across 8 cores (8192 each). Each core:
  K-side: per 128-col chunk, compute (W@K_chunk)^T in PSUM via
    matmul(lhsT=K_chunk, rhs=W^T), exp with per-partition bias
    (A + ln D_norm - |k|^2) -> phiK^T [128, 256]; accumulate
    kv^T [64, 256] += V_chunk^T @ phiK^T in PSUM over all 64 chunks.
  AllReduce kv^T (64KB) across the 8 cores (overlapped with Q-side features).
  Q-side: rawQ = exp(A + ln D_norm + B*(W@Q)) [256, L] with W^T stationary
    (bf16 output), |q|^2 via ones-matmul; out_chunk [128, 64] =
    rawQ_chunk^T @ kv, scaled per-partition by exp(-|q|^2).
"""
import numpy as np

N_CORES = 8
D = 64
L = 65536
R = 256
DV = 64
LS = L // N_CORES          # 8192 per core
TILE = 2048                # streaming tile of L
N_TILES = LS // TILE       # 4
CPT = TILE // 128          # chunks per tile = 16
N_CHUNKS = LS // 128       # 64
QB = 512                   # q feature block
N_QB = LS // QB            # 16

A_CONST = 0.1
S_CONST = 1.0
B_CONST = (S_CONST * (1.0 - 4.0 * A_CONST)) ** 0.5
LN_DNORM = (D / 4.0) * float(np.log(1.0 - 4.0 * A_CONST))
BIAS_CONST = A_CONST + LN_DNORM          # exp bias for both sides
# both phi calls use c = -1.0 (C_CONST*S_CONST and C_CONST)

_CACHE = {}


def _build(reps=1, collective=True):
    import concourse.bass as bass
    import concourse.bacc as bacc
    import concourse.tile as tile
    import concourse.mybir as mybir

    f32 = mybir.dt.float32
    f32r = mybir.dt.float32r
    bf16 = mybir.dt.bfloat16

    nc = bacc.Bacc("TRN2", target_bir_lowering=False, debug=False,
                   num_devices=N_CORES)
    k_ext = nc.dram_tensor("K", [D, LS], f32r, kind="ExternalInput").ap()
    q_ext = nc.dram_tensor("Q", [D, LS], f32r, kind="ExternalInput").ap()
    v_ext = nc.dram_tensor("Vr", [128, N_CHUNKS * DV], f32r,
                           kind="ExternalInput").ap()
    wt_ext = nc.dram_tensor("Wt", [D, R + 2], f32r, kind="ExternalInput").ap()
    i64_ext = nc.dram_tensor("I64", [64, 64], f32, kind="ExternalInput").ap()
    out_ext = nc.dram_tensor("out", [LS, DV], f32, kind="ExternalOutput").ap()

    import contextlib

    with tile.TileContext(nc) as tc:
        loop_cm = tc.For_i(0, reps, 1) if reps != 1 else contextlib.nullcontext()
        with (
            loop_cm,
            tc.tile_pool(name="const", bufs=1) as cpool,
            tc.tile_pool(name="kq", bufs=3) as kq_pool,
            tc.tile_pool(name="x2", bufs=2) as x2_pool,
            tc.tile_pool(name="phikt", bufs=4) as phikt_pool,
            tc.tile_pool(name="rawq", bufs=2 * N_QB) as rawq_pool,
            tc.tile_pool(name="small", bufs=10) as small_pool,
            tc.tile_pool(name="outsb", bufs=4) as out_pool,
            tc.tile_pool(name="ps_feat", bufs=2, space="PSUM") as ps_feat,
            tc.tile_pool(name="ps_q", bufs=2, space="PSUM") as ps_q,
            tc.tile_pool(name="ps_kvt", bufs=1, space="PSUM") as ps_kvt,
            tc.tile_pool(name="ps_sq", bufs=1, space="PSUM") as ps_sq,
            tc.tile_pool(name="ps_out", bufs=2, space="PSUM") as ps_out,
            tc.tile_pool(name="dram", bufs=1, space="DRAM") as dram,
        ):
            # constants
            wt_sb = cpool.tile([D, R + 2], f32r, name="wt_sb")
            nc.sync.dma_start(wt_sb[:], wt_ext[:])
            i64_sb = cpool.tile([64, 64], f32, name="i64_sb")
            nc.sync.dma_start(i64_sb[:], i64_ext[:])
            biasq_sb = cpool.tile([128, 1], f32, name="biasq_sb")
            nc.vector.memset(biasq_sb[:], BIAS_CONST)
            zero_sb = cpool.tile([128, 1], f32, name="zero_sb")
            nc.vector.memset(zero_sb[:], 0.0)
            v_sb = cpool.tile([128, N_CHUNKS * DV], f32r, name="v_sb")
            nc.sync.dma_start(v_sb[:], v_ext[:])

            wt_r = wt_sb[:, 0:R]
            ones_r = wt_sb[:, R:R + 2]

            kvt_ps = ps_kvt.tile([64, R], f32)   # kv^T accumulator

            # ---------------- K side ----------------
            for t in range(N_TILES):
                k_t = kq_pool.tile([D, TILE], f32r, tag="kq")
                nc.sync.dma_start(k_t[:], k_ext[:, t * TILE:(t + 1) * TILE])
                k2_t = x2_pool.tile([D, TILE], f32r, tag="x2")
                nc.vector.tensor_mul(k2_t[:], k_t[:], k_t[:])
                sq_ps = ps_sq.tile([128, 2 * CPT], f32, tag="sq")
                for j in range(CPT):
                    nc.tensor.matmul(
                        sq_ps[:, 2 * j:2 * j + 2],
                        k2_t[:, j * 128:(j + 1) * 128],
                        ones_r,
                        start=True, stop=True,
                    )
                sq_view = sq_ps[:].rearrange("p (c two) -> p c two", two=2)[:, :, 0]
                bias_t = small_pool.tile([128, CPT], f32, tag="biask")
                nc.vector.tensor_scalar(
                    bias_t[:], sq_view, -1.0, BIAS_CONST,
                    mybir.AluOpType.mult, mybir.AluOpType.add,
                )
                for j in range(CPT):
                    i = t * CPT + j
                    feat_ps = ps_feat.tile([128, R], f32, tag="feat")
                    nc.tensor.matmul(
                        feat_ps[:],
                        k_t[:, j * 128:(j + 1) * 128],
                        wt_r,
                        start=True, stop=True,
                    )
                    phikt = phikt_pool.tile([128, R], f32r, tag="phikt")
                    nc.scalar.activation(
                        phikt[:], feat_ps[:],
                        mybir.ActivationFunctionType.Exp,
                        bias=bias_t[:, j:j + 1], scale=B_CONST,
                    )
                    nc.tensor.matmul(
                        kvt_ps[:],
                        v_sb[:, i * DV:(i + 1) * DV],
                        phikt[:],
                        start=(i == 0), stop=(i == N_CHUNKS - 1),
                    )

            # ---------------- kv^T AllReduce ----------------
            kvt_sb = cpool.tile([64, R], f32, name="kvt_sb")
            nc.vector.tensor_copy(kvt_sb[:], kvt_ps[:])
            cc_in = dram.tile([64, R], f32)
            cc_out = dram.tile([64, R], f32)
            nc.sync.dma_start(cc_in[:], kvt_sb[:])
            if collective:
                nc.gpsimd.collective_compute(
                    "AllReduce",
                    mybir.AluOpType.add,
                    ins=[cc_in.opt()],
                    outs=[cc_out.opt()],
                    replica_groups=[list(range(N_CORES))],
                )
            else:
                nc.sync.dma_start(cc_out[:], cc_in[:])

            # ---------------- Q side features (overlap collective) --------
            rawq = []   # [h][b] -> [128, 512] bf16
            sq_q_tiles = []
            for t in range(N_TILES):
                q_t = kq_pool.tile([D, TILE], f32r, tag="kq")
                nc.sync.dma_start(q_t[:], q_ext[:, t * TILE:(t + 1) * TILE])
                q2_t = x2_pool.tile([D, TILE], f32r, tag="x2")
                nc.vector.tensor_mul(q2_t[:], q_t[:], q_t[:])
                sq_ps = ps_sq.tile([128, 2 * CPT], f32, tag="sq")
                for j in range(CPT):
                    nc.tensor.matmul(
                        sq_ps[:, 2 * j:2 * j + 2],
                        q2_t[:, j * 128:(j + 1) * 128],
                        ones_r,
                        start=True, stop=True,
                    )
                sq_view = sq_ps[:].rearrange("p (c two) -> p c two", two=2)[:, :, 0]
                sq_t = small_pool.tile([128, CPT], f32, tag="sqq")
                nc.scalar.activation(
                    sq_t[:], sq_view,
                    mybir.ActivationFunctionType.Exp,
                    bias=zero_sb[:], scale=-1.0,
                )
                sq_q_tiles.append(sq_t)
                for b in range(TILE // QB):
                    for h in range(2):
                        q_ps = ps_q.tile([128, QB], f32, tag="q")
                        nc.tensor.matmul(
                            q_ps[:],
                            wt_sb[:, h * 128:(h + 1) * 128],
                            q_t[:, b * QB:(b + 1) * QB],
                            start=True, stop=True,
                        )
                        rq = rawq_pool.tile([128, QB], bf16, tag="rawq")
                        nc.scalar.activation(
                            rq[:], q_ps[:],
                            mybir.ActivationFunctionType.Exp,
                            bias=biasq_sb[:], scale=B_CONST,
                        )
                        rawq.append(rq)

            # ---------------- kv back + transpose ----------------
            kvt_red = cpool.tile([64, R], f32, name="kvt_red")
            nc.sync.dma_start(kvt_red[:], cc_out[:])
            kv_bf = []
            for h in range(2):
                kv_ps = ps_out.tile([128, DV], f32, tag="out")
                nc.tensor.transpose(
                    kv_ps[:], kvt_red[:, h * 128:(h + 1) * 128], i64_sb[:])
                kvh = cpool.tile([128, DV], bf16, name=f"kv_bf{h}")
                nc.vector.tensor_copy(kvh[:], kv_ps[:])
                kv_bf.append(kvh)

            # ---------------- out = rawQ^T @ kv, scaled ----------------
            for i in range(N_CHUNKS):
                gb = i // 4              # global 512-block
                s = (i % 4) * 128
                o_ps = ps_out.tile([128, DV], f32, tag="out")
                nc.tensor.matmul(
                    o_ps[:], rawq[2 * gb][:, s:s + 128], kv_bf[0][:],
                    start=True, stop=False,
                )
                nc.tensor.matmul(
                    o_ps[:], rawq[2 * gb + 1][:, s:s + 128], kv_bf[1][:],
                    start=False, stop=True,
                )
                o_sb = out_pool.tile([128, DV], f32, tag="osb")
                sq_t = sq_q_tiles[i // CPT]
                nc.vector.tensor_scalar(
                    o_sb[:], o_ps[:], sq_t[:, (i % CPT):(i % CPT) + 1], None,
                    mybir.AluOpType.mult,
                )
                nc.sync.dma_start(
                    out_ext[i * 128:(i + 1) * 128, :], o_sb[:])

    nc.compile()
    return nc


def _get_nc(reps=1, collective=True):
    key = (reps, collective)
    if key not in _CACHE:
        _CACHE[key] = _build(reps=reps, collective=collective)
    return _CACHE[key]


def _make_in_maps(Q, K, V, W):
    wt = np.ascontiguousarray(
        np.concatenate([W.T, np.ones((D, 2))], axis=1).astype(np.float32))
    i64 = np.eye(64, dtype=np.float32)
    maps = []
    for c in range(N_CORES):
        sl = slice(c * LS, (c + 1) * LS)
        vr = np.ascontiguousarray(
            V[sl].reshape(N_CHUNKS, 128, DV).transpose(1, 0, 2)
            .reshape(128, N_CHUNKS * DV).astype(np.float32))
        maps.append({
            "Q": np.ascontiguousarray(Q[:, sl].astype(np.float32)),
            "K": np.ascontiguousarray(K[:, sl].astype(np.float32)),
            "Vr": vr,
            "Wt": wt,
            "I64": i64,
        })
    return maps


def _run(in_maps):
    from concourse.bass_utils import run_bass_kernel_spmd
    nc = _get_nc()
    return run_bass_kernel_spmd(nc, in_maps, list(range(N_CORES)))


def kernel(Q, K, V, W):
    res = _run(_make_in_maps(Q, K, V, W))
    out = np.concatenate([res.results[c]["out"] for c in range(N_CORES)],
                         axis=0)
    return out.astype(np.float32)
